# revision 15
# baseline (speedup 1.0000x reference)
"""Causal depthwise conv1d (B=8, C=1024, T=8192, K=4, dil=1) on 8 trn2 cores.

Sharding: batch-parallel — core j handles x[j] (1024, 8192), communication-free.

Per-core kernel (Bass/Tile):
  - channels -> 8 partition blocks of 128; time -> 4 chunks of 2048 (+3 halo)
  - work split per 512-col psum group to keep every engine under the DMA
    roofline (~1.3us per group):
      PE:  taps 1..3 as fp32r matmuls with lhsT = diag(w[:,k]), rhs = the x
           tile shifted by k in the free dim, accumulated in one PSUM bank
      ACT: tap 0 fused with bias: tmp = x0 * w0 + bias (per-partition
           scale/bias APs)
      DVE: out = tmp + psum (tensor_tensor add), evicting PSUM
  - HBM traffic is the roofline: 32 MiB in + 32 MiB out per core.
"""
import numpy as np

import concourse.bacc as bacc
import concourse.mybir as mybir
from concourse.tile import TileContext
from concourse.tile import add_dep_helper
from concourse import bass_utils

B, C, T, K = 8, 1024, 8192, 4
HALO = K - 1          # causal left pad
P = 128               # SBUF partitions
RBLK = C // P         # 8 channel blocks per core
CHUNK = 2048          # time chunk per inner iteration
IOBUFS = 4            # xt pool bufs
OTBUFS = 6            # ot pool bufs (slot-reuse distance for the WAR dep)
NCHUNK = T // CHUNK   # 4
NGRP = CHUNK // 512   # psum groups per chunk
NPE = K - 1           # taps done on PE (1..3); tap 0 rides the ACT pass

_cached = {}


def _build():
    nc = bacc.Bacc("TRN2", target_bir_lowering=False, debug=False)
    f32 = mybir.dt.float32
    f32r = mybir.dt.float32r

    x_d = nc.dram_tensor("x", [C, T], f32r, kind="ExternalInput")
    wd_d = nc.dram_tensor("wd", [P, RBLK * NPE * P], f32r, kind="ExternalInput")
    w0_d = nc.dram_tensor("w0", [P, RBLK], f32, kind="ExternalInput")
    w1_d = nc.dram_tensor("w1", [P, RBLK], f32, kind="ExternalInput")
    b_d = nc.dram_tensor("bv", [P, RBLK], f32, kind="ExternalInput")
    y_d = nc.dram_tensor("y", [C, T], f32, kind="ExternalOutput")

    with TileContext(nc) as tc:
        with (
            tc.tile_pool(name="const", bufs=1) as cpool,
            tc.tile_pool(name="io", bufs=IOBUFS) as pool,
            tc.tile_pool(name="ox", bufs=OTBUFS) as opool,
            tc.tile_pool(name="tmp", bufs=8) as tpool,
            tc.tile_pool(name="psum", bufs=8, space="PSUM") as psum_pool,
        ):
            wt = cpool.tile([P, RBLK * NPE * P], f32r)
            nc.gpsimd.dma_start(out=wt, in_=wd_d.ap())
            w0t = cpool.tile([P, RBLK], f32)
            nc.gpsimd.dma_start(out=w0t, in_=w0_d.ap())
            w1t = cpool.tile([P, RBLK], f32)
            nc.sync.dma_start(out=w1t, in_=w1_d.ap())
            bt = cpool.tile([P, RBLK], f32)
            nc.gpsimd.dma_start(out=bt, in_=b_d.ap())

            # ot-slot store DMAs ride the ACT HWDGE ring (parallel to the SP
            # ring carrying loads). Tile misses the WAR edge "store complete
            # before DVE reuses the slot" for ACT-issued DMAs (it credits
            # ACT program order with completion), so add it explicitly.
            store_insts = []
            for r in range(RBLK):
                rows = slice(r * P, (r + 1) * P)
                for i in range(NCHUNK):
                    n = r * NCHUNK + i
                    xt = pool.tile([P, CHUNK + HALO], f32r, tag="xt")
                    if i == 0:
                        # memset doesn't support f32r; zero via uint32 view
                        nc.vector.memset(xt[:, 0:HALO].bitcast(mybir.dt.uint32), 0)
                        nc.sync.dma_start(out=xt[:, HALO:],
                                          in_=x_d.ap()[rows, 0:CHUNK])
                    else:
                        nc.sync.dma_start(
                            out=xt,
                            in_=x_d.ap()[rows, i * CHUNK - HALO:(i + 1) * CHUNK])
                    xf = xt.bitcast(f32)

                    ot = opool.tile([P, CHUNK], f32, tag="ot")
                    for s in range(NGRP):
                        ps = psum_pool.tile([P, 512], f32)
                        for k in range(1, K):
                            nc.tensor.matmul(
                                ps,
                                wt[:, (r * NPE + k - 1) * P:(r * NPE + k) * P],
                                xt[:, s * 512 + k:s * 512 + k + 512],
                                start=(k == 1), stop=(k == K - 1))
                        tmp = tpool.tile([P, 512], f32, tag="tmp")
                        nc.scalar.activation(
                            tmp, xf[:, s * 512:s * 512 + 512],
                            mybir.ActivationFunctionType.Identity,
                            bias=bt[:, r:r + 1], scale=w0t[:, r:r + 1])
                        tt = nc.vector.tensor_add(
                            out=ot[:, s * 512:(s + 1) * 512], in0=tmp, in1=ps)
                        if s == 0 and n >= OTBUFS:
                            add_dep_helper(
                                tt.ins, store_insts[n - OTBUFS].ins,
                                reason="ot slot reuse waits for store DMA")
                    st = nc.scalar.dma_start(
                        out=y_d.ap()[rows, i * CHUNK:(i + 1) * CHUNK], in_=ot)
                    store_insts.append(st)
    nc.compile()
    return nc


def _host_weights(w, b):
    # wd[p, (r*NPE+k-1)*P + m] = w[r*P+m, 0, k] if p == m else 0 (lhsT diags,
    # taps 1..K-1); tap 0 is applied by the ACT pass via w0.
    wd = np.zeros((P, RBLK * NPE * P), dtype=np.float32)
    m = np.arange(P)
    for r in range(RBLK):
        for k in range(1, K):
            wd[m, (r * NPE + k - 1) * P + m] = w[r * P + m, 0, k]
    w0 = np.ascontiguousarray(w[:, 0, 0].reshape(RBLK, P).T).astype(np.float32)
    w1 = np.ascontiguousarray(w[:, 0, 1].reshape(RBLK, P).T).astype(np.float32)
    bv = np.ascontiguousarray(b.reshape(RBLK, P).T).astype(np.float32)
    return wd, w0, w1, bv


def kernel(x, w, b):
    x = np.asarray(x, dtype=np.float32)
    w = np.asarray(w, dtype=np.float32)
    b = np.asarray(b, dtype=np.float32)

    if "nc" not in _cached:
        _cached["nc"] = _build()
    nc = _cached["nc"]

    wd, w0, w1, bv = _host_weights(w, b)
    in_maps = [
        {"x": np.ascontiguousarray(x[j]), "wd": wd, "w0": w0, "w1": w1,
         "bv": bv}
        for j in range(B)
    ]
    res = bass_utils.run_bass_kernel_spmd(nc, in_maps, core_ids=list(range(B)))
    return np.stack([r["y"] for r in res.results], axis=0)


# revision 16
# speedup vs baseline: 1.1188x; 1.1188x over previous
"""Causal depthwise conv1d (B=8, C=1024, T=8192, K=4, dil=1) on 8 trn2 cores.

Sharding: batch-parallel — core j handles x[j] (1024, 8192), communication-free.

Per-core kernel (Bass/Tile):
  - channels -> 8 partition blocks of 128; time -> 4 chunks of 2048 (+3 halo)
  - work split per 512-col psum group to keep every engine under the DMA
    roofline (~1.3us per group):
      PE:  taps 1..3 as fp32r matmuls with lhsT = diag(w[:,k]), rhs = the x
           tile shifted by k in the free dim, accumulated in one PSUM bank
      ACT: tap 0 fused with bias: tmp = x0 * w0 + bias (per-partition
           scale/bias APs)
      DVE: out = tmp + psum (tensor_tensor add), evicting PSUM
  - HBM traffic is the roofline: 32 MiB in + 32 MiB out per core.
"""
import numpy as np

import concourse.bacc as bacc
import concourse.mybir as mybir
from concourse.tile import TileContext
from concourse.tile import add_dep_helper
from concourse import bass_utils

B, C, T, K = 8, 1024, 8192, 4
HALO = K - 1          # causal left pad
P = 128               # SBUF partitions
RBLK = C // P         # 8 channel blocks per core
CHUNK = 2048          # time chunk per inner iteration
IOBUFS = 4            # xt pool bufs
OTBUFS = 6            # ot pool bufs (slot-reuse distance for the WAR dep)
NCHUNK = T // CHUNK   # 4
NGRP = CHUNK // 512   # psum groups per chunk
NPE = K - 1           # taps done on PE (1..3); tap 0 rides the ACT pass

_cached = {}


def _build():
    nc = bacc.Bacc("TRN2", target_bir_lowering=False, debug=False)
    f32 = mybir.dt.float32
    f32r = mybir.dt.float32r

    x_d = nc.dram_tensor("x", [C, T], f32r, kind="ExternalInput")
    wd_d = nc.dram_tensor("wd", [P, RBLK * NPE * P], f32r, kind="ExternalInput")
    w0_d = nc.dram_tensor("w0", [P, RBLK], f32, kind="ExternalInput")
    w1_d = nc.dram_tensor("w1", [P, RBLK], f32, kind="ExternalInput")
    b_d = nc.dram_tensor("bv", [P, RBLK], f32, kind="ExternalInput")
    y_d = nc.dram_tensor("y", [C, T], f32, kind="ExternalOutput")

    with TileContext(nc) as tc:
        with (
            tc.tile_pool(name="const", bufs=1) as cpool,
            tc.tile_pool(name="io", bufs=IOBUFS) as pool,
            tc.tile_pool(name="ox", bufs=OTBUFS) as opool,
            tc.tile_pool(name="tmp", bufs=8) as tpool,
            tc.tile_pool(name="psum", bufs=8, space="PSUM") as psum_pool,
        ):
            wt = cpool.tile([P, RBLK * NPE * P], f32r)
            nc.sync.dma_start(out=wt, in_=wd_d.ap())
            w0t = cpool.tile([P, RBLK], f32)
            nc.sync.dma_start(out=w0t, in_=w0_d.ap())
            w1t = cpool.tile([P, RBLK], f32)
            nc.sync.dma_start(out=w1t, in_=w1_d.ap())
            bt = cpool.tile([P, RBLK], f32)
            nc.sync.dma_start(out=bt, in_=b_d.ap())

            # ot-slot store DMAs ride the ACT HWDGE ring (parallel to the SP
            # ring carrying loads). Tile misses the WAR edge "store complete
            # before DVE reuses the slot" for ACT-issued DMAs (it credits
            # ACT program order with completion), so add it explicitly.
            store_insts = []
            for r in range(RBLK):
                rows = slice(r * P, (r + 1) * P)
                for i in range(NCHUNK):
                    n = r * NCHUNK + i
                    xt = pool.tile([P, CHUNK + HALO], f32r, tag="xt")
                    if i == 0:
                        # memset doesn't support f32r; zero via uint32 view
                        nc.vector.memset(xt[:, 0:HALO].bitcast(mybir.dt.uint32), 0)
                        nc.sync.dma_start(out=xt[:, HALO:],
                                          in_=x_d.ap()[rows, 0:CHUNK])
                    else:
                        nc.sync.dma_start(
                            out=xt,
                            in_=x_d.ap()[rows, i * CHUNK - HALO:(i + 1) * CHUNK])
                    xf = xt.bitcast(f32)

                    ot = opool.tile([P, CHUNK], f32, tag="ot")
                    for s in range(NGRP):
                        ps = psum_pool.tile([P, 512], f32)
                        for k in range(1, K):
                            nc.tensor.matmul(
                                ps,
                                wt[:, (r * NPE + k - 1) * P:(r * NPE + k) * P],
                                xt[:, s * 512 + k:s * 512 + k + 512],
                                start=(k == 1), stop=(k == K - 1))
                        tmp = tpool.tile([P, 512], f32, tag="tmp")
                        nc.scalar.activation(
                            tmp, xf[:, s * 512:s * 512 + 512],
                            mybir.ActivationFunctionType.Identity,
                            bias=bt[:, r:r + 1], scale=w0t[:, r:r + 1])
                        tt = nc.vector.tensor_add(
                            out=ot[:, s * 512:(s + 1) * 512], in0=tmp, in1=ps)
                        if s == 0 and n >= OTBUFS:
                            add_dep_helper(
                                tt.ins, store_insts[n - OTBUFS].ins,
                                reason="ot slot reuse waits for store DMA")
                    st = nc.scalar.dma_start(
                        out=y_d.ap()[rows, i * CHUNK:(i + 1) * CHUNK], in_=ot)
                    store_insts.append(st)
    nc.compile()
    return nc


def _host_weights(w, b):
    # wd[p, (r*NPE+k-1)*P + m] = w[r*P+m, 0, k] if p == m else 0 (lhsT diags,
    # taps 1..K-1); tap 0 is applied by the ACT pass via w0.
    wd = np.zeros((P, RBLK * NPE * P), dtype=np.float32)
    m = np.arange(P)
    for r in range(RBLK):
        for k in range(1, K):
            wd[m, (r * NPE + k - 1) * P + m] = w[r * P + m, 0, k]
    w0 = np.ascontiguousarray(w[:, 0, 0].reshape(RBLK, P).T).astype(np.float32)
    w1 = np.ascontiguousarray(w[:, 0, 1].reshape(RBLK, P).T).astype(np.float32)
    bv = np.ascontiguousarray(b.reshape(RBLK, P).T).astype(np.float32)
    return wd, w0, w1, bv


def kernel(x, w, b):
    x = np.asarray(x, dtype=np.float32)
    w = np.asarray(w, dtype=np.float32)
    b = np.asarray(b, dtype=np.float32)

    if "nc" not in _cached:
        _cached["nc"] = _build()
    nc = _cached["nc"]

    wd, w0, w1, bv = _host_weights(w, b)
    in_maps = [
        {"x": np.ascontiguousarray(x[j]), "wd": wd, "w0": w0, "w1": w1,
         "bv": bv}
        for j in range(B)
    ]
    res = bass_utils.run_bass_kernel_spmd(nc, in_maps, core_ids=list(range(B)))
    return np.stack([r["y"] for r in res.results], axis=0)


# revision 18
# speedup vs baseline: 1.1197x; 1.0008x over previous
"""Causal depthwise conv1d (B=8, C=1024, T=8192, K=4, dil=1) on 8 trn2 cores.

Sharding: batch-parallel — core j handles x[j] (1024, 8192), communication-free.

Per-core kernel (Bass/Tile):
  - channels -> 8 partition blocks of 128; time -> 4 chunks of 2048 (+3 halo)
  - work split per 512-col psum group to keep every engine under the DMA
    roofline (~1.3us per group):
      PE:  taps 1..3 as fp32r matmuls with lhsT = diag(w[:,k]), rhs = the x
           tile shifted by k in the free dim, accumulated in one PSUM bank
      ACT: tap 0 fused with bias: tmp = x0 * w0 + bias (per-partition
           scale/bias APs)
      DVE: out = tmp + psum (tensor_tensor add), evicting PSUM
  - HBM traffic is the roofline: 32 MiB in + 32 MiB out per core.
"""
import numpy as np

import concourse.bacc as bacc
import concourse.mybir as mybir
from concourse.tile import TileContext
from concourse.tile import add_dep_helper
from concourse import bass_utils

B, C, T, K = 8, 1024, 8192, 4
HALO = K - 1          # causal left pad
P = 128               # SBUF partitions
RBLK = C // P         # 8 channel blocks per core
CHUNK = 2048          # time chunk per inner iteration
IOBUFS = 4            # xt pool bufs
OTBUFS = 8            # ot pool bufs (slot-reuse distance for the WAR dep)
NCHUNK = T // CHUNK   # 4
NGRP = CHUNK // 512   # psum groups per chunk
NPE = K - 1           # taps done on PE (1..3); tap 0 rides the ACT pass

_cached = {}


def _build():
    nc = bacc.Bacc("TRN2", target_bir_lowering=False, debug=False)
    f32 = mybir.dt.float32
    f32r = mybir.dt.float32r

    x_d = nc.dram_tensor("x", [C, T], f32r, kind="ExternalInput")
    wd_d = nc.dram_tensor("wd", [P, RBLK * NPE * P], f32r, kind="ExternalInput")
    w0_d = nc.dram_tensor("w0", [P, RBLK], f32, kind="ExternalInput")
    b_d = nc.dram_tensor("bv", [P, RBLK], f32, kind="ExternalInput")
    y_d = nc.dram_tensor("y", [C, T], f32, kind="ExternalOutput")

    with TileContext(nc) as tc:
        with (
            tc.tile_pool(name="const", bufs=1) as cpool,
            tc.tile_pool(name="io", bufs=IOBUFS) as pool,
            tc.tile_pool(name="ox", bufs=OTBUFS) as opool,
            tc.tile_pool(name="tmp", bufs=8) as tpool,
            tc.tile_pool(name="psum", bufs=8, space="PSUM") as psum_pool,
        ):
            wt = cpool.tile([P, RBLK * NPE * P], f32r)
            nc.sync.dma_start(out=wt, in_=wd_d.ap())
            w0t = cpool.tile([P, RBLK], f32)
            nc.sync.dma_start(out=w0t, in_=w0_d.ap())
            bt = cpool.tile([P, RBLK], f32)
            nc.sync.dma_start(out=bt, in_=b_d.ap())

            # ot-slot store DMAs ride the ACT HWDGE ring (parallel to the SP
            # ring carrying loads). Tile misses the WAR edge "store complete
            # before DVE reuses the slot" for ACT-issued DMAs (it credits
            # ACT program order with completion), so add it explicitly.
            store_insts = []
            for r in range(RBLK):
                rows = slice(r * P, (r + 1) * P)
                for i in range(NCHUNK):
                    n = r * NCHUNK + i
                    xt = pool.tile([P, CHUNK + HALO], f32r, tag="xt")
                    if i == 0:
                        # memset doesn't support f32r; zero via uint32 view
                        nc.vector.memset(xt[:, 0:HALO].bitcast(mybir.dt.uint32), 0)
                        nc.sync.dma_start(out=xt[:, HALO:],
                                          in_=x_d.ap()[rows, 0:CHUNK])
                    else:
                        nc.sync.dma_start(
                            out=xt,
                            in_=x_d.ap()[rows, i * CHUNK - HALO:(i + 1) * CHUNK])
                    xf = xt.bitcast(f32)

                    ot = opool.tile([P, CHUNK], f32, tag="ot")
                    for s in range(NGRP):
                        ps = psum_pool.tile([P, 512], f32)
                        for k in range(1, K):
                            nc.tensor.matmul(
                                ps,
                                wt[:, (r * NPE + k - 1) * P:(r * NPE + k) * P],
                                xt[:, s * 512 + k:s * 512 + k + 512],
                                start=(k == 1), stop=(k == K - 1))
                        tmp = tpool.tile([P, 512], f32, tag="tmp")
                        nc.scalar.activation(
                            tmp, xf[:, s * 512:s * 512 + 512],
                            mybir.ActivationFunctionType.Identity,
                            bias=bt[:, r:r + 1], scale=w0t[:, r:r + 1])
                        tt = nc.vector.tensor_add(
                            out=ot[:, s * 512:(s + 1) * 512], in0=tmp, in1=ps)
                        if s == 0 and n >= OTBUFS:
                            add_dep_helper(
                                tt.ins, store_insts[n - OTBUFS].ins,
                                reason="ot slot reuse waits for store DMA")
                    st = nc.scalar.dma_start(
                        out=y_d.ap()[rows, i * CHUNK:(i + 1) * CHUNK], in_=ot)
                    store_insts.append(st)
    nc.compile()
    return nc


def _host_weights(w, b):
    # wd[p, (r*NPE+k-1)*P + m] = w[r*P+m, 0, k] if p == m else 0 (lhsT diags,
    # taps 1..K-1); tap 0 is applied by the ACT pass via w0.
    wd = np.zeros((P, RBLK * NPE * P), dtype=np.float32)
    m = np.arange(P)
    for r in range(RBLK):
        for k in range(1, K):
            wd[m, (r * NPE + k - 1) * P + m] = w[r * P + m, 0, k]
    w0 = np.ascontiguousarray(w[:, 0, 0].reshape(RBLK, P).T).astype(np.float32)
    bv = np.ascontiguousarray(b.reshape(RBLK, P).T).astype(np.float32)
    return wd, w0, bv


def kernel(x, w, b):
    x = np.asarray(x, dtype=np.float32)
    w = np.asarray(w, dtype=np.float32)
    b = np.asarray(b, dtype=np.float32)

    if "nc" not in _cached:
        _cached["nc"] = _build()
    nc = _cached["nc"]

    wd, w0, bv = _host_weights(w, b)
    in_maps = [
        {"x": np.ascontiguousarray(x[j]), "wd": wd, "w0": w0, "bv": bv}
        for j in range(B)
    ]
    res = bass_utils.run_bass_kernel_spmd(nc, in_maps, core_ids=list(range(B)))
    return np.stack([r["y"] for r in res.results], axis=0)


# revision 19
# speedup vs baseline: 1.1283x; 1.0077x over previous
"""Causal depthwise conv1d (B=8, C=1024, T=8192, K=4, dil=1) on 8 trn2 cores.

Sharding: batch-parallel — core j handles x[j] (1024, 8192), communication-free.

Per-core kernel (Bass/Tile):
  - channels -> 8 partition blocks of 128; time -> 4 chunks of 2048 (+3 halo)
  - work split per 512-col psum group to keep every engine under the DMA
    roofline (~1.3us per group):
      PE:  taps 1..3 as fp32r matmuls with lhsT = diag(w[:,k]), rhs = the x
           tile shifted by k in the free dim, accumulated in one PSUM bank
      ACT: tap 0 fused with bias: tmp = x0 * w0 + bias (per-partition
           scale/bias APs)
      DVE: out = tmp + psum (tensor_tensor add), evicting PSUM
  - HBM traffic is the roofline: 32 MiB in + 32 MiB out per core.
"""
import numpy as np

import concourse.bacc as bacc
import concourse.mybir as mybir
from concourse.tile import TileContext
from concourse.tile import add_dep_helper
from concourse import bass_utils

B, C, T, K = 8, 1024, 8192, 4
HALO = K - 1          # causal left pad
P = 128               # SBUF partitions
RBLK = C // P         # 8 channel blocks per core
CHUNK = 2048          # time chunk per inner iteration
IOBUFS = 5            # xt pool bufs
OTBUFS = 8            # ot pool bufs (slot-reuse distance for the WAR dep)
NCHUNK = T // CHUNK   # 4
NGRP = CHUNK // 512   # psum groups per chunk
NPE = K - 1           # taps done on PE (1..3); tap 0 rides the ACT pass

_cached = {}


def _build():
    nc = bacc.Bacc("TRN2", target_bir_lowering=False, debug=False)
    f32 = mybir.dt.float32
    f32r = mybir.dt.float32r

    x_d = nc.dram_tensor("x", [C, T], f32r, kind="ExternalInput")
    wd_d = nc.dram_tensor("wd", [P, RBLK * NPE * P], f32r, kind="ExternalInput")
    w0_d = nc.dram_tensor("w0", [P, RBLK], f32, kind="ExternalInput")
    b_d = nc.dram_tensor("bv", [P, RBLK], f32, kind="ExternalInput")
    y_d = nc.dram_tensor("y", [C, T], f32, kind="ExternalOutput")

    with TileContext(nc) as tc:
        with (
            tc.tile_pool(name="const", bufs=1) as cpool,
            tc.tile_pool(name="io", bufs=IOBUFS) as pool,
            tc.tile_pool(name="ox", bufs=OTBUFS) as opool,
            tc.tile_pool(name="tmp", bufs=8) as tpool,
            tc.tile_pool(name="psum", bufs=8, space="PSUM") as psum_pool,
        ):
            wt = cpool.tile([P, RBLK * NPE * P], f32r)
            nc.sync.dma_start(out=wt, in_=wd_d.ap())
            w0t = cpool.tile([P, RBLK], f32)
            nc.sync.dma_start(out=w0t, in_=w0_d.ap())
            bt = cpool.tile([P, RBLK], f32)
            nc.sync.dma_start(out=bt, in_=b_d.ap())

            # ot-slot store DMAs ride the ACT HWDGE ring (parallel to the SP
            # ring carrying loads). Tile misses the WAR edge "store complete
            # before DVE reuses the slot" for ACT-issued DMAs (it credits
            # ACT program order with completion), so add it explicitly.
            store_insts = []
            for r in range(RBLK):
                rows = slice(r * P, (r + 1) * P)
                for i in range(NCHUNK):
                    n = r * NCHUNK + i
                    xt = pool.tile([P, CHUNK + HALO], f32r, tag="xt")
                    if i == 0:
                        # memset doesn't support f32r; zero via uint32 view
                        nc.vector.memset(xt[:, 0:HALO].bitcast(mybir.dt.uint32), 0)
                        nc.sync.dma_start(out=xt[:, HALO:],
                                          in_=x_d.ap()[rows, 0:CHUNK])
                    else:
                        nc.sync.dma_start(
                            out=xt,
                            in_=x_d.ap()[rows, i * CHUNK - HALO:(i + 1) * CHUNK])
                    xf = xt.bitcast(f32)

                    ot = opool.tile([P, CHUNK], f32, tag="ot")
                    for s in range(NGRP):
                        ps = psum_pool.tile([P, 512], f32)
                        for k in range(1, K):
                            nc.tensor.matmul(
                                ps,
                                wt[:, (r * NPE + k - 1) * P:(r * NPE + k) * P],
                                xt[:, s * 512 + k:s * 512 + k + 512],
                                start=(k == 1), stop=(k == K - 1))
                        tmp = tpool.tile([P, 512], f32, tag="tmp")
                        nc.scalar.activation(
                            tmp, xf[:, s * 512:s * 512 + 512],
                            mybir.ActivationFunctionType.Identity,
                            bias=bt[:, r:r + 1], scale=w0t[:, r:r + 1])
                        tt = nc.vector.tensor_add(
                            out=ot[:, s * 512:(s + 1) * 512], in0=tmp, in1=ps)
                        if s == 0 and n >= OTBUFS:
                            add_dep_helper(
                                tt.ins, store_insts[n - OTBUFS].ins,
                                reason="ot slot reuse waits for store DMA")
                    st = nc.scalar.dma_start(
                        out=y_d.ap()[rows, i * CHUNK:(i + 1) * CHUNK], in_=ot)
                    store_insts.append(st)
    nc.compile()
    return nc


def _host_weights(w, b):
    # wd[p, (r*NPE+k-1)*P + m] = w[r*P+m, 0, k] if p == m else 0 (lhsT diags,
    # taps 1..K-1); tap 0 is applied by the ACT pass via w0.
    wd = np.zeros((P, RBLK * NPE * P), dtype=np.float32)
    m = np.arange(P)
    for r in range(RBLK):
        for k in range(1, K):
            wd[m, (r * NPE + k - 1) * P + m] = w[r * P + m, 0, k]
    w0 = np.ascontiguousarray(w[:, 0, 0].reshape(RBLK, P).T).astype(np.float32)
    bv = np.ascontiguousarray(b.reshape(RBLK, P).T).astype(np.float32)
    return wd, w0, bv


def kernel(x, w, b):
    x = np.asarray(x, dtype=np.float32)
    w = np.asarray(w, dtype=np.float32)
    b = np.asarray(b, dtype=np.float32)

    if "nc" not in _cached:
        _cached["nc"] = _build()
    nc = _cached["nc"]

    wd, w0, bv = _host_weights(w, b)
    in_maps = [
        {"x": np.ascontiguousarray(x[j]), "wd": wd, "w0": w0, "bv": bv}
        for j in range(B)
    ]
    res = bass_utils.run_bass_kernel_spmd(nc, in_maps, core_ids=list(range(B)))
    return np.stack([r["y"] for r in res.results], axis=0)
